# revision 1
# baseline (speedup 1.0000x reference)
"""Multi-head causal attention with RoPE on 8 trn2 NeuronCores.

Problem (hardcoded): B=2, S=2048, D=2048, H=16, Hd=128, fp32.
  q/k/v = x @ wq/wk/wv; RoPE(q,k); causal softmax(q k^T/sqrt(Hd)) @ v; out @ wo.

Sharding: core c = 4*b + g handles batch b, heads [4g, 4g+4).
  - wq/wk/wv column-parallel (512 cols per core); wo column-parallel fed by
    per-q-quarter AllGathers of the local attention outputs o^T inside each
    batch group of 4.
  - x and all weights are cast to fp16 on the host: fp16 matmuls run 1
    cycle/row (same as fp32r) but halve DMA bytes and SBUF footprint, fp16
    PE transposes run 1.0 instead of 1.5 cycles/row, and all accumulation
    stays fp32 in PSUM (measured end-to-end max rel err ~4e-4).
  - Host-side prep: per-head even/odd column permutation of wq/wk makes RoPE
    partition-aligned in the transposed [head_dim, S] layout.
  - The causal mask (and the widened dead zone of the last diagonal block)
    is applied by a PE matmul accumulating a +/-1e30 constant into the score
    PSUM tile, keeping the DVE off the scores->exp->pv critical chain.
  - PSUM banks (8): A family = q-projection accumulators / vps / pv(2) /
    dn(1) / yps, B family = k-projection / transposes / sc(3) / yps, with
    phase-3 yps alternating A3/B3 so the output projection overlaps
    attention.
  - Softmax denominators are accumulated on the DVE (exp tiles summed into
    an SBUF accumulator, one ones-matmul per (q-block, head) at the end),
    saving ~64k PE cycles of per-k-block ones-matmuls. The DVE is used --
    not the Pool -- because collectives block the Pool queue.
  - o^T is AllGathered fp16 in four S/4 quarters, each fired right after its
    attention block; the output projection of quarter j-1 is emitted between
    attention blocks j and j+1 so phase 3 fills phase-2 pipeline stalls.
    wo (fp16) is prefetched on the software-DGE queue at t=0.
  - NOTE: do NOT use dma_start(transpose=True) (InstDmaTransposeAnt) here:
    its completion ordering is unreliable on this runtime and corrupts
    consumers nondeterministically. PE transposes via identity matmul are
    used instead.
"""
import math
import numpy as np

import concourse.bass as bass
import concourse.tile as tile
from concourse import bacc, mybir
from concourse.bass_utils import run_bass_kernel_spmd

F32 = mybir.dt.float32
F32R = mybir.dt.float32r
F16 = mybir.dt.float16
EXPF = mybir.ActivationFunctionType.Exp
ADD = mybir.AluOpType.add
MULT = mybir.AluOpType.mult

B, S, D = 2, 2048, 2048
H, HD = 16, 128
HPC = 4              # heads per core
DC = HPC * HD        # 512 d_out per core
NCHUNK = D // 128    # 16 contraction chunks
SB = 512             # s-block (projection and q-block granularity)
NSB = S // SB        # 4
SCALE = 1.0 / math.sqrt(HD)
NEG = -1.0e30

RG = [[0, 1, 2, 3], [4, 5, 6, 7]]


def build_module(trace_sim=False, phases=(1, 2, 3), repeat=1):
    nc = bacc.Bacc("TRN2", target_bir_lowering=False, debug=False, num_devices=8)

    x = nc.dram_tensor("x", [S, D], F16, kind="ExternalInput").ap()
    wq = nc.dram_tensor("wq", [D, DC], F16, kind="ExternalInput").ap()
    wk = nc.dram_tensor("wk", [D, DC], F16, kind="ExternalInput").ap()
    wv = nc.dram_tensor("wv", [D, DC], F16, kind="ExternalInput").ap()
    wo = nc.dram_tensor("wo", [D, DC], F16, kind="ExternalInput").ap()
    c2 = nc.dram_tensor("c2", [128, S], F32, kind="ExternalInput").ap()
    s2n = nc.dram_tensor("s2n", [128, S], F32, kind="ExternalInput").ap()
    tri = nc.dram_tensor("tri", [128, 128], F32R, kind="ExternalInput").ap()
    trid = nc.dram_tensor("trid", [128, 256], F32R, kind="ExternalInput").ap()
    idr = nc.dram_tensor("idr", [128, 128], F32R, kind="ExternalInput").ap()
    ones = nc.dram_tensor("ones", [128, 128], F32R, kind="ExternalInput").ap()
    ident = nc.dram_tensor("ident", [128, 128], F16, kind="ExternalInput").ap()
    y = nc.dram_tensor("y", [S, DC], F32, kind="ExternalOutput").ap()

    ot_loc = [nc.dram_tensor(f"ot_loc{i}", [DC, SB], F16) for i in range(4)]
    ot_full = [nc.dram_tensor(f"ot_full{i}", [D, SB], F16) for i in range(4)]

    with tile.TileContext(nc, trace_sim=trace_sim) as tc:
        with tc.tile_pool(name="consts", bufs=1) as cpool:
            ones_t = cpool.tile([128, 128], F32R)
            nc.sync.dma_start(ones_t[:], ones[:])
            tri_t = cpool.tile([128, 128], F32R)
            nc.sync.dma_start(tri_t[:], tri[:])
            trid_t = cpool.tile([128, 256], F32R)
            nc.sync.dma_start(trid_t[:], trid[:])
            idr_t = cpool.tile([128, 128], F32R)
            nc.sync.dma_start(idr_t[:], idr[:])
            c2_t = cpool.tile([128, S], F32)
            nc.sync.dma_start(c2_t[:], c2[:])
            s2n_t = cpool.tile([128, S], F32)
            nc.sync.dma_start(s2n_t[:], s2n[:])
            id_t = cpool.tile([128, 128], F16)
            nc.sync.dma_start(id_t[:], ident[:])
            # wo prefetch on the software-DGE (Pool) queue: off the hwdge
            # queues that feed phase 1, resident by the time phase 3 starts.
            wo_t = [cpool.tile([128, DC], F16, name=f"wo{c}")
                    for c in range(NCHUNK)]
            for c in range(NCHUNK):
                nc.gpsimd.dma_start(wo_t[c][:], wo[c * 128:(c + 1) * 128, :])
            cst = dict(ones_t=ones_t, tri_t=tri_t, c2_t=c2_t, s2n_t=s2n_t,
                       id_t=id_t, trid_t=trid_t, idr_t=idr_t)

            for rep in range(repeat):
                with tc.tile_pool(name=f"qkres{rep}", bufs=1) as qkpool, \
                     tc.tile_pool(name=f"vres{rep}", bufs=1) as vpool, \
                     tc.tile_pool(name=f"p1sb{rep}", bufs=2) as p1, \
                     tc.tile_pool(name=f"p1xt{rep}", bufs=1) as p1x, \
                     tc.tile_pool(name=f"p2sb{rep}", bufs=3) as p2, \
                     tc.tile_pool(name=f"p3sb{rep}", bufs=2) as p3, \
                     tc.tile_pool(name=f"ps{rep}", bufs=1, space="PSUM") as ps:
                    qt_res = [qkpool.tile([128, S], F16, name=f"qt{h}")
                              for h in range(HPC)]
                    kt_res = [qkpool.tile([128, S], F16, name=f"kt{h}")
                              for h in range(HPC)]
                    v_t = [vpool.tile([128, DC], F32R, name=f"v{kb}")
                           for kb in range(S // 128)]

                    for j in range(NSB):
                        _p1_block(nc, j, x, wq, wk, wv, v_t, qt_res, kt_res,
                                  p1, p1x, ps, cst)
                    if 2 in phases:
                        for j in range(NSB):
                            _p2_block(nc, j, v_t, ot_loc, qt_res, kt_res,
                                      p2, ps, cst)
                            if 3 in phases:
                                if j == 3:
                                    _p3_block(nc, 2, ot_full, wo_t, y,
                                              p3, ps)
                                nc.gpsimd.collective_compute(
                                    "AllGather", mybir.AluOpType.bypass,
                                    replica_groups=RG,
                                    ins=[ot_loc[j][:]], outs=[ot_full[j][:]])
                                # output projection of q-quarter j-1
                                # overlaps attention block j+1
                                if 1 <= j <= 2:
                                    _p3_block(nc, j - 1, ot_full, wo_t, y,
                                              p3, ps)
                        if 3 in phases:
                            _p3_block(nc, 3, ot_full, wo_t, y, p3, ps)

    nc.compile()
    return nc


def _p1_block(nc, j, x, wq, wk, wv, v_t, qt_res, kt_res, p1, p1x, ps, cst):
    """Projection + RoPE for s-block j: qt/kt slices [j*SB,(j+1)*SB), v blocks."""
    s0 = j * SB
    NH = NCHUNK // 2
    xt2 = [p1x.tile([128, NH * SB], F16, tag=f"xt{half}",
                    name=f"xt_{j}_{half}") for half in range(2)]

    def xt(c):
        return xt2[c // NH][:, (c % NH) * SB:(c % NH + 1) * SB]

    for ss in range(SB // 128):
        xrow = p1.tile([128, D], F16, tag="xrow", bufs=3)
        nc.gpsimd.dma_start(
            xrow[:, 0:D // 2], x[s0 + ss * 128:s0 + (ss + 1) * 128, 0:D // 2])
        nc.scalar.dma_start(
            xrow[:, D // 2:], x[s0 + ss * 128:s0 + (ss + 1) * 128, D // 2:])
        for c4 in range(NCHUNK // 4):
            tp = ps.tile([128, 512], F16, tag=f"B{c4 % 2}", bufs=1,
                         name=f"tp{j}_{ss}_{c4}")
            for cc in range(4):
                c = c4 * 4 + cc
                nc.tensor.transpose(
                    tp[:, cc * 128:(cc + 1) * 128],
                    xrow[:, c * 128:(c + 1) * 128], cst["id_t"][:])
            half = c4 // 2
            NH = NCHUNK // 2
            dst = xt2[half][:].rearrange("p (c f) -> p c f", c=NH)[
                :, (c4 % 2) * 4:(c4 % 2) * 4 + 4, ss * 128:ss * 128 + 128]
            src_ = tp[:].rearrange("p (c f) -> p c f", c=4)
            if c4 % 2 == 0:
                nc.scalar.copy(dst, src_)
            else:
                nc.vector.tensor_copy(dst, src_)

    # q-pass then k-pass: chunk-outer weight streaming, 4 held accumulators
    for (wsrc, res_list, wtag) in ((wq, qt_res, "wq"), (wk, kt_res, "wk")):
        fam = "A" if wtag == "wq" else "B"
        prj = [ps.tile([128, SB], F32, tag=f"{fam}{h}", bufs=1,
                       name=f"prj{wtag}{j}_{h}") for h in range(HPC)]
        for c in range(NCHUNK):
            wt = p1.tile([128, DC], F16, tag=wtag, bufs=5,
                         name=f"{wtag}t{j}_{c}")
            nc.sync.dma_start(wt[:], wsrc[c * 128:(c + 1) * 128, :])
            for h in range(HPC):
                nc.tensor.matmul(
                    prj[h][:], wt[:, h * 128:(h + 1) * 128],
                    xt(c),
                    start=(c == 0), stop=(c == NCHUNK - 1))
        for h in range(HPC):
            raw = p1.tile([128, SB], F32, tag="rraw", bufs=1)
            nc.scalar.copy(raw[:], prj[h][:])
            swp = p1.tile([128, SB], F32, tag="rswp", bufs=1)
            nc.gpsimd.dma_start(swp[0:64, :], raw[64:128, :])
            nc.gpsimd.dma_start(swp[64:128, :], raw[0:64, :])
            t1 = p1.tile([128, SB], F32, tag="rt1", bufs=1)
            nc.vector.tensor_tensor(
                t1[:], raw[:], cst["c2_t"][:, s0:s0 + SB], op=MULT)
            t2 = p1.tile([128, SB], F32, tag="rt2", bufs=1)
            nc.vector.tensor_tensor(
                t2[:], swp[:], cst["s2n_t"][:, s0:s0 + SB], op=MULT)
            nc.vector.tensor_tensor(
                res_list[h][:, s0:s0 + SB], t1[:], t2[:], op=ADD)

    # v-pass (natural layout), chunk-outer, into resident v_t
    vps = [ps.tile([128, DC], F32, tag=f"A{ss}", bufs=1,
                   name=f"vps{j}_{ss}") for ss in range(SB // 128)]
    for c in range(NCHUNK):
        wt = p1.tile([128, DC], F16, tag="wv", bufs=5, name=f"wvt{j}_{c}")
        nc.sync.dma_start(wt[:], wv[c * 128:(c + 1) * 128, :])
        for ss in range(SB // 128):
            nc.tensor.matmul(
                vps[ss][:], xt(c)[:, ss * 128:(ss + 1) * 128],
                wt[:], start=(c == 0), stop=(c == NCHUNK - 1))
    for ss in range(SB // 128):
        nc.vector.tensor_copy(v_t[j * 4 + ss][:], vps[ss][:])


def _p2_block(nc, j, v_t, ot_loc, qt_res, kt_res, p2, ps, cst):
    """Causal attention for q-block j (keys/values blocks 0..4j+3)."""
    q0 = j * SB
    nkb = 4 * (j + 1)
    for hp in range(HPC // 2):
        heads = (2 * hp, 2 * hp + 1)
        pv = {h: ps.tile([128, SB], F32, tag=f"A{h % 2}",
                         name=f"pv{j}_{h}") for h in heads}
        # softmax denominators: accumulate exp tiles on the DVE (off the
        # Pool queue, which collectives block) and reduce once per head
        eac = {h: p2.tile([128, SB], F32R, tag=f"eac{h % 2}", bufs=2,
                          name=f"eac{j}_{h}") for h in heads}
        for kb in range(nkb):
            r = kb - 4 * j
            if r < 0:
                lo = 0
            elif r <= 2:
                lo = r * 128
            else:
                lo = 256
            for h in heads:
                sc = ps.tile([128, SB], F32, tag=f"B{(kb * 2 + h) % 3}",
                             name=f"sc{j}_{h}_{kb}")
                nc.tensor.matmul(
                    sc[:, lo:], kt_res[h][:, kb * 128:(kb + 1) * 128],
                    qt_res[h][:, q0 + lo:q0 + SB],
                    start=True, stop=(r < 0), skip_group_check=True)
                if r == 3:
                    # widened dead zone + diagonal mask in one accumulation
                    nc.tensor.matmul(
                        sc[:, 256:], cst["idr_t"][:], cst["trid_t"][:],
                        start=False, stop=True, skip_group_check=True)
                elif r >= 0:
                    nc.tensor.matmul(
                        sc[:, r * 128:(r + 1) * 128], cst["idr_t"][:],
                        cst["tri_t"][:],
                        start=False, stop=True, skip_group_check=True)
                ep = p2.tile([128, SB], F32R, tag="ep", bufs=4)
                nc.scalar.activation(ep[:, lo:], sc[:, lo:], EXPF, scale=SCALE)
                if kb == 0:
                    nc.vector.tensor_copy(eac[h][:], ep[:])
                else:
                    nc.vector.tensor_tensor(
                        eac[h][:, lo:], eac[h][:, lo:], ep[:, lo:], op=ADD)
                nc.tensor.matmul(
                    pv[h][:, lo:], v_t[kb][:, h * 128:(h + 1) * 128],
                    ep[:, lo:],
                    start=(kb == 0), stop=(kb == nkb - 1),
                    skip_group_check=True)
        for h in heads:
            dn = ps.tile([128, SB], F32, tag="A2", name=f"dn{j}_{h}")
            nc.tensor.matmul(dn[:], cst["ones_t"][:], eac[h][:],
                             start=True, stop=True)
            rec = p2.tile([128, SB], F32, tag="rec", bufs=2)
            nc.vector.reciprocal(rec[:], dn[:])
            ot = p2.tile([128, SB], F16, tag="ot", bufs=2)
            nc.vector.tensor_tensor(ot[:], pv[h][:], rec[:], op=MULT)
            nc.sync.dma_start(
                ot_loc[j][h * 128:(h + 1) * 128, :], ot[:])


def _p3_block(nc, sq, ot_full, wo_t, y, p3, ps):
    """Output projection for q-quarter sq from its gathered o^T (fp16)."""
    o0 = sq * 512
    otf = [p3.tile([128, 512], F16, tag=f"otf{c}", bufs=2,
                   name=f"otf{c}_{sq}") for c in range(NCHUNK)]
    for c in range(NCHUNK):
        nc.sync.dma_start(
            otf[c][:], ot_full[sq][c * 128:(c + 1) * 128, :])
    for ss in range(4):
        yps = ps.tile([128, DC], F32, tag=("A3" if ss % 2 == 0 else "B3"),
                      name=f"yps{sq}_{ss}")
        for c in range(NCHUNK):
            nc.tensor.matmul(
                yps[:], otf[c][:, ss * 128:(ss + 1) * 128],
                wo_t[c][:], start=(c == 0), stop=(c == NCHUNK - 1))
        ysb = p3.tile([128, DC], F32, tag="ysb")
        nc.vector.tensor_copy(ysb[:], yps[:])
        nc.sync.dma_start(
            y[o0 + ss * 128:o0 + (ss + 1) * 128, :], ysb[:])


_PERM = np.concatenate([np.arange(0, 128, 2), np.arange(1, 128, 2)])


def _f16(a):
    return np.ascontiguousarray(a).astype(np.float16)


def make_in_maps(x, wq, wk, wv, wo, freqs_cos, freqs_sin):
    """Host-side sharding/prep. Returns list of 8 per-core input dicts."""
    cosT = np.ascontiguousarray(freqs_cos.T.astype(np.float32))   # [64, S]
    sinT = np.ascontiguousarray(freqs_sin.T.astype(np.float32))
    c2 = np.concatenate([cosT, cosT], axis=0)                     # [128, S]
    s2n = np.concatenate([-sinT, sinT], axis=0)
    tri = np.where(np.arange(128)[None, :] >= np.arange(128)[:, None],
                   0.0, NEG).astype(np.float32)                   # [k, q]
    ones = np.ones((128, 128), dtype=np.float32)
    ident = np.eye(128, dtype=np.float16)
    idr = np.eye(128, dtype=np.float32)
    trid = np.concatenate([np.full((128, 128), NEG, np.float32),
                           np.where(np.arange(128)[None, :] >=
                                    np.arange(128)[:, None], 0.0,
                                    NEG).astype(np.float32)], axis=1)

    in_maps = []
    for c in range(8):
        b, g = divmod(c, 4)
        cols = slice(g * DC, (g + 1) * DC)
        wq_c = np.ascontiguousarray(wq[:, cols]).copy()
        wk_c = np.ascontiguousarray(wk[:, cols]).copy()
        for h in range(HPC):
            blk = slice(h * 128, (h + 1) * 128)
            wq_c[:, blk] = wq_c[:, blk][:, _PERM]
            wk_c[:, blk] = wk_c[:, blk][:, _PERM]
        in_maps.append({
            "x": _f16(x[b]),
            "wq": _f16(wq_c),
            "wk": _f16(wk_c),
            "wv": _f16(wv[:, cols]),
            "wo": _f16(wo[:, cols]),
            "c2": c2, "s2n": s2n, "tri": tri, "ones": ones, "ident": ident,
            "idr": idr, "trid": trid,
        })
    return in_maps


def assemble(results):
    """Concatenate per-core column outputs into [B, S, D]."""
    out = np.empty((B, S, D), dtype=np.float32)
    for c in range(8):
        b, g = divmod(c, 4)
        out[b][:, g * DC:(g + 1) * DC] = results[c]["y"]
    return out


_NC = None


def kernel(x, wq, wk, wv, wo, freqs_cos, freqs_sin):
    global _NC
    x = np.asarray(x); wq = np.asarray(wq); wk = np.asarray(wk)
    wv = np.asarray(wv); wo = np.asarray(wo)
    freqs_cos = np.asarray(freqs_cos); freqs_sin = np.asarray(freqs_sin)
    if _NC is None:
        _NC = build_module()
    in_maps = make_in_maps(x, wq, wk, wv, wo, freqs_cos, freqs_sin)
    res = run_bass_kernel_spmd(_NC, in_maps, core_ids=list(range(8)))
    return assemble(res.results)



# revision 2
# speedup vs baseline: 2.3273x; 2.3273x over previous
"""Multi-head causal attention with RoPE on 8 trn2 NeuronCores (v2).

Problem (hardcoded): B=2, S=2048, D=2048, H=16, Hd=128, fp32.
  q/k/v = x @ wq/wk/wv; RoPE(q,k); causal softmax(q k^T/sqrt(Hd)) @ v; out @ wo.

Sharding: core c = 4*b + g handles batch b, heads [4g, 4g+4).
Changes vs v1 (see git history in docstring of kernel.py):
  - x is transposed on the HOST (xt = x[b].T, fp16): kills all 64 PE
    transposes, the xt assembly copies, and the xrow DMAs.
  - wq/wk/wv/wo are fully SBUF-resident, DMA'd once at t=0 in 4-chunk
    groups (vs re-streamed every s-block): -18MB HBM traffic, -176 DMA
    dispatch stubs.
  - Causal masks are fp16 (NEG=-30000) instead of fp32r: mask matmuls
    run 1 cycle/row instead of 4 (sub-256-wide fp32r penalty).
  - RoPE runs fully in fp16 on the DVE at the 2x perf mode; the
    partition swap is done with output-base-offset tensor_tensor ops
    (hw-verified) instead of SBUF-to-SBUF DMAs: no Pool-queue load.
  - ep (exp of scores) is fp16; eac (softmax denominator accumulator)
    is fp16 at DVE 2x / first-copy 4x.
  - ysb copies split between DVE and Act to balance engine queues.
"""
import math
import numpy as np

import concourse.bass as bass
import concourse.tile as tile
from concourse import bacc, mybir
from concourse.bass_utils import run_bass_kernel_spmd

F32 = mybir.dt.float32
F32R = mybir.dt.float32r
F16 = mybir.dt.float16
EXPF = mybir.ActivationFunctionType.Exp
ADD = mybir.AluOpType.add
MULT = mybir.AluOpType.mult

B, S, D = 2, 2048, 2048
H, HD = 16, 128
HPC = 4              # heads per core
DC = HPC * HD        # 512 d_out per core
NCHUNK = D // 128    # 16 contraction chunks
NG = NCHUNK // 4     # 4 chunk-groups of 4
SB = 512             # s-block (projection and q-block granularity)
NSB = S // SB        # 4
SCALE = 1.0 / math.sqrt(HD)
NEG = -30000.0       # fp16-safe; exp(scale*(s+NEG)) == 0

RG = [[0, 1, 2, 3], [4, 5, 6, 7]]


def build_module(trace_sim=False, phases=(1, 2, 3), repeat=1):
    nc = bacc.Bacc("TRN2", target_bir_lowering=False, debug=False, num_devices=8)

    xt = nc.dram_tensor("xt", [D, S], F16, kind="ExternalInput").ap()
    wq = nc.dram_tensor("wq", [D, DC], F16, kind="ExternalInput").ap()
    wk = nc.dram_tensor("wk", [D, DC], F16, kind="ExternalInput").ap()
    wv = nc.dram_tensor("wv", [D, DC], F16, kind="ExternalInput").ap()
    wo = nc.dram_tensor("wo", [D, DC], F16, kind="ExternalInput").ap()
    c2 = nc.dram_tensor("c2", [128, S], F16, kind="ExternalInput").ap()
    s2sw = nc.dram_tensor("s2sw", [128, S], F16, kind="ExternalInput").ap()
    tri = nc.dram_tensor("tri", [128, 128], F16, kind="ExternalInput").ap()
    trid = nc.dram_tensor("trid", [128, 256], F16, kind="ExternalInput").ap()
    idr = nc.dram_tensor("idr", [128, 128], F16, kind="ExternalInput").ap()
    ones = nc.dram_tensor("ones", [128, 128], F16, kind="ExternalInput").ap()
    y = nc.dram_tensor("y", [S, DC], F32, kind="ExternalOutput").ap()

    ot_loc = [nc.dram_tensor(f"ot_loc{i}", [DC, SB], F16) for i in range(4)]
    ot_full = [nc.dram_tensor(f"ot_full{i}", [D, SB], F16) for i in range(3)]
    # quarter 3 is gathered in two head-pair halves so the first AllGather
    # hides under the tail of p2(3): piece rows = [core][head 2h..2h+2][hd]
    ot_f3 = [nc.dram_tensor(f"ot_f3{p}", [D // 2, SB], F16) for p in range(2)]

    with tile.TileContext(nc, trace_sim=trace_sim) as tc:
        with tc.tile_pool(name="consts", bufs=1) as cpool:
            # resident weights, 4-chunk groups: w4[g][:, t*DC + dc] is
            # chunk 4g+t. DMA issuance is deferred into the rep loop so
            # the j=0 x tiles can be first in every queue's FIFO.
            def wload(ap_, name, engine, engine0=None):
                tiles = []
                for g in range(NG):
                    t_ = cpool.tile([128, 4 * DC], F16, name=f"{name}{g}")
                    src = ap_[g * 512:(g + 1) * 512, :].rearrange(
                        "(t p) d -> p t d", t=4)
                    dst = t_[:].rearrange("p (t d) -> p t d", t=4)
                    eng = engine0 if (g == 0 and engine0 is not None) else engine
                    eng.dma_start(dst, src)
                    tiles.append(t_)
                return tiles

            def load_consts():
                c2_t = cpool.tile([128, S], F16)
                nc.scalar.dma_start(c2_t[:], c2[:])
                s2_t = cpool.tile([128, S], F16)
                nc.scalar.dma_start(s2_t[:], s2sw[:])
                tri_t = cpool.tile([128, 128], F16)
                nc.gpsimd.dma_start(tri_t[:], tri[:])
                trid_t = cpool.tile([128, 256], F16)
                nc.gpsimd.dma_start(trid_t[:], trid[:])
                idr_t = cpool.tile([128, 128], F16)
                nc.gpsimd.dma_start(idr_t[:], idr[:])
                ones_t = cpool.tile([128, 128], F16)
                nc.gpsimd.dma_start(ones_t[:], ones[:])
                return dict(ones_t=ones_t, tri_t=tri_t, c2_t=c2_t,
                            s2_t=s2_t, idr_t=idr_t, trid_t=trid_t)

            cst = wq_t = wk_t = wv_t = wo_t = None
            for rep in range(repeat):
                with tc.tile_pool(name=f"qkres{rep}", bufs=1) as qkpool, \
                     tc.tile_pool(name=f"vres{rep}", bufs=1) as vpool, \
                     tc.tile_pool(name=f"p1sb{rep}", bufs=2) as p1, \
                     tc.tile_pool(name=f"p1xt{rep}", bufs=2) as p1x, \
                     tc.tile_pool(name=f"p2sb{rep}", bufs=3) as p2, \
                     tc.tile_pool(name=f"p3sb{rep}", bufs=2) as p3, \
                     tc.tile_pool(name=f"ps{rep}", bufs=1, space="PSUM") as ps:
                    qt_res = [qkpool.tile([128, S], F16, name=f"qt{h}")
                              for h in range(HPC)]
                    kt_res = [qkpool.tile([128, S], F16, name=f"kt{h}")
                              for h in range(HPC)]
                    v_t = [vpool.tile([128, DC], F16, name=f"v{kb}")
                           for kb in range(S // 128)]

                    # j=0 x tiles first in the sync/scalar FIFOs, then the
                    # resident weights (wq earliest: the q-pass streams it
                    # chunk-group by chunk-group), then everything else.
                    xt_pre = _xload(nc, 0, xt, p1x)
                    if rep == 0:
                        wq_t = wload(wq, "wq", nc.sync, engine0=nc.gpsimd)
                        cst = load_consts()
                        wk_t = wload(wk, "wk", nc.scalar)
                        wv_t = wload(wv, "wv", nc.gpsimd)
                        wo_t = wload(wo, "wo", nc.gpsimd)

                    for j in range(NSB):
                        _p1_block(nc, j, xt, wq_t, wk_t, wv_t, v_t,
                                  qt_res, kt_res, p1, p1x, ps, cst,
                                  xt_pre if j == 0 else None)
                    def ag(src_ap, dst_ap):
                        nc.gpsimd.collective_compute(
                            "AllGather", mybir.AluOpType.bypass,
                            replica_groups=RG, ins=[src_ap], outs=[dst_ap])

                    if 2 in phases:
                        for j in range(NSB):
                            cb = None
                            if j == 3 and 3 in phases:
                                def cb(hp):
                                    if hp == 0:
                                        ag(ot_loc[3][0:DC // 2, :],
                                           ot_f3[0][:])
                            _p2_block(nc, j, v_t, ot_loc, qt_res, kt_res,
                                      p2, ps, cst, on_hp=cb)
                            if 3 in phases:
                                if j == 3:
                                    _p3_block(nc, 2, ot_full, wo_t, y,
                                              p3, ps)
                                    ag(ot_loc[3][DC // 2:, :], ot_f3[1][:])
                                else:
                                    ag(ot_loc[j][:], ot_full[j][:])
                                if 1 <= j <= 2:
                                    _p3_block(nc, j - 1, ot_full, wo_t, y,
                                              p3, ps)
                        if 3 in phases:
                            _p3_block3(nc, ot_f3, wo_t, y, p3, ps)

    nc.compile()
    return nc


def _xload(nc, j, xt, p1x):
    s0 = j * SB
    xt4 = [p1x.tile([128, 4 * SB], F16, tag=f"xt{g}", name=f"xt_{j}_{g}",
                    bufs=(2 if g < 2 else 1))
           for g in range(NG)]
    for g in range(NG):
        src = xt[g * 512:(g + 1) * 512, s0:s0 + SB].rearrange(
            "(t p) s -> p t s", t=4)
        dst = xt4[g][:].rearrange("p (t s) -> p t s", t=4)
        (nc.sync if g % 2 == 0 else nc.scalar).dma_start(dst, src)
    return xt4


def _p1_block(nc, j, xt, wq_t, wk_t, wv_t, v_t, qt_res, kt_res, p1, p1x, ps,
              cst, xt_pre=None):
    """Projection + RoPE for s-block j from host-transposed x."""
    s0 = j * SB
    xt4 = xt_pre if xt_pre is not None else _xload(nc, j, xt, p1x)

    def xsl(c, lo=0, width=SB):
        g, t = divmod(c, 4)
        return xt4[g][:, t * SB + lo:t * SB + lo + width]

    # q-pass then k-pass: 4 held PSUM accumulators, resident weights
    for (w4, res_list, fam) in ((wq_t, qt_res, "A"), (wk_t, kt_res, "B")):
        prj = [ps.tile([128, SB], F32, tag=f"{fam}{h}", bufs=1,
                       name=f"prj{fam}{j}_{h}") for h in range(HPC)]
        for c in range(NCHUNK):
            g, t = divmod(c, 4)
            for h in range(HPC):
                nc.tensor.matmul(
                    prj[h][:], w4[g][:, t * DC + h * 128:t * DC + (h + 1) * 128],
                    xsl(c), start=(c == 0), stop=(c == NCHUNK - 1))
        for h in range(HPC):
            raw = p1.tile([128, SB], F16, tag="rraw", bufs=2)
            nc.scalar.copy(raw[:], prj[h][:])
            t1 = p1.tile([128, SB], F16, tag="rt1", bufs=2)
            nc.vector.tensor_tensor(
                t1[:], raw[:], cst["c2_t"][:, s0:s0 + SB], op=MULT)
            t2 = p1.tile([128, SB], F16, tag="rt2", bufs=2)
            # partition swap via output-base offset (hw-verified):
            # t2[p] = raw[p^64] * s2sw[p^64]
            nc.vector.tensor_tensor(
                t2[0:64, :], raw[64:128, :], cst["s2_t"][64:128, s0:s0 + SB],
                op=MULT)
            nc.vector.tensor_tensor(
                t2[64:128, :], raw[0:64, :], cst["s2_t"][0:64, s0:s0 + SB],
                op=MULT)
            nc.vector.tensor_tensor(
                res_list[h][:, s0:s0 + SB], t1[:], t2[:], op=ADD)

    # v-pass (natural layout), x-stationary, into resident v_t
    vps = [ps.tile([128, DC], F32, tag=f"A{ss}", bufs=1,
                   name=f"vps{j}_{ss}") for ss in range(SB // 128)]
    for c in range(NCHUNK):
        g, t = divmod(c, 4)
        for ss in range(SB // 128):
            nc.tensor.matmul(
                vps[ss][:], xsl(c, ss * 128, 128),
                wv_t[g][:, t * DC:(t + 1) * DC],
                start=(c == 0), stop=(c == NCHUNK - 1))
    for ss in range(SB // 128):
        nc.vector.tensor_copy(v_t[j * 4 + ss][:], vps[ss][:])


def _p2_block(nc, j, v_t, ot_loc, qt_res, kt_res, p2, ps, cst, on_hp=None):
    """Causal attention for q-block j (keys/values blocks 0..4j+3)."""
    q0 = j * SB
    nkb = 4 * (j + 1)
    for hp in range(HPC // 2):
        heads = (2 * hp, 2 * hp + 1)
        pv = {h: ps.tile([128, SB], F32, tag=f"A{h % 2}",
                         name=f"pv{j}_{h}") for h in heads}
        eac = {h: p2.tile([128, SB], F16, tag=f"eac{h % 2}", bufs=2,
                          name=f"eac{j}_{h}") for h in heads}
        for kb in range(nkb):
            r = kb - 4 * j
            if r < 0:
                lo = 0
            elif r <= 2:
                lo = r * 128
            else:
                lo = 256
            for h in heads:
                sc = ps.tile([128, SB], F32, tag=f"B{(kb * 2 + h) % 3}",
                             name=f"sc{j}_{h}_{kb}")
                nc.tensor.matmul(
                    sc[:, lo:], kt_res[h][:, kb * 128:(kb + 1) * 128],
                    qt_res[h][:, q0 + lo:q0 + SB],
                    start=True, stop=(r < 0), skip_group_check=True)
                if r == 3:
                    nc.tensor.matmul(
                        sc[:, 256:], cst["idr_t"][:], cst["trid_t"][:],
                        start=False, stop=True, skip_group_check=True)
                elif r >= 0:
                    nc.tensor.matmul(
                        sc[:, r * 128:(r + 1) * 128], cst["idr_t"][:],
                        cst["tri_t"][:],
                        start=False, stop=True, skip_group_check=True)
                ep = p2.tile([128, SB], F16, tag="ep", bufs=4)
                nc.scalar.activation(ep[:, lo:], sc[:, lo:], EXPF, scale=SCALE)
                if kb == 0:
                    nc.vector.tensor_copy(eac[h][:], ep[:])
                else:
                    nc.vector.tensor_tensor(
                        eac[h][:, lo:], eac[h][:, lo:], ep[:, lo:], op=ADD)
                nc.tensor.matmul(
                    pv[h][:, lo:], v_t[kb][:, h * 128:(h + 1) * 128],
                    ep[:, lo:],
                    start=(kb == 0), stop=(kb == nkb - 1),
                    skip_group_check=True)
        for h in heads:
            dn = ps.tile([128, SB], F32, tag="A2", name=f"dn{j}_{h}")
            nc.tensor.matmul(dn[:], cst["ones_t"][:], eac[h][:],
                             start=True, stop=True)
            rec = p2.tile([128, SB], F32, tag="rec", bufs=2)
            nc.vector.reciprocal(rec[:], dn[:])
            ot = p2.tile([128, SB], F16, tag="ot", bufs=2)
            nc.vector.tensor_tensor(ot[:], pv[h][:], rec[:], op=MULT)
            nc.sync.dma_start(
                ot_loc[j][h * 128:(h + 1) * 128, :], ot[:])
        if on_hp is not None:
            on_hp(hp)


def _p3_block(nc, sq, ot_full, wo_t, y, p3, ps):
    """Output projection for q-quarter sq from its gathered o^T."""
    o0 = sq * 512
    otf = [p3.tile([128, 4 * SB], F16, tag=f"otf{g}", bufs=1,
                   name=f"otf{g}_{sq}") for g in range(NG)]
    for g in range(NG):
        src = ot_full[sq][g * 512:(g + 1) * 512, :].rearrange(
            "(t p) s -> p t s", t=4)
        dst = otf[g][:].rearrange("p (t s) -> p t s", t=4)
        # sync only: stubs on the Act queue would delay the p2 exp chain
        nc.sync.dma_start(dst, src)
    for ss in range(4):
        yps = ps.tile([128, DC], F32, tag=("A3" if ss % 2 == 0 else "B3"),
                      name=f"yps{sq}_{ss}")
        for c in range(NCHUNK):
            g, t = divmod(c, 4)
            nc.tensor.matmul(
                yps[:], otf[g][:, t * SB + ss * 128:t * SB + (ss + 1) * 128],
                wo_t[g][:, t * DC:(t + 1) * DC],
                start=(c == 0), stop=(c == NCHUNK - 1))
        ysb = p3.tile([128, DC], F32, tag="ysb")
        nc.vector.tensor_copy(ysb[:], yps[:])
        nc.sync.dma_start(
            y[o0 + ss * 128:o0 + (ss + 1) * 128, :], ysb[:])


def _p3_block3(nc, ot_f3, wo_t, y, p3, ps):
    """Output projection for q-quarter 3 from the two head-pair-half
    gathers. Piece p chunk cp (= core*2 + h-in-pair) maps to original
    o-dim chunk (cp//2)*4 + 2p + cp%2 — a pure index remap on wo_t."""
    o0 = 3 * 512
    otf = {}
    for p in range(2):
        for gg in range(2):
            t_ = p3.tile([128, 4 * SB], F16, tag=f"otf{p * 2 + gg}", bufs=1,
                         name=f"otf3_{p}_{gg}")
            src = ot_f3[p][gg * 512:(gg + 1) * 512, :].rearrange(
                "(t p) s -> p t s", t=4)
            dst = t_[:].rearrange("p (t s) -> p t s", t=4)
            (nc.sync if gg == 0 else nc.scalar).dma_start(dst, src)
            otf[(p, gg)] = t_
    # piece-major: all piece-0 accumulation (hides under the second
    # AllGather), then all piece-1. Four yps banks held concurrently.
    yps = [ps.tile([128, DC], F32, tag=t, name=f"yps3_{i}")
           for i, t in enumerate(("A3", "B3", "A0", "A1"))]
    for p in range(2):
        for ss in range(4):
            for cp in range(8):
                gg, tt = divmod(cp, 4)
                wc = (cp // 2) * 4 + 2 * p + cp % 2
                wg, wt_ = divmod(wc, 4)
                nc.tensor.matmul(
                    yps[ss][:],
                    otf[(p, gg)][:, tt * SB + ss * 128:tt * SB + (ss + 1) * 128],
                    wo_t[wg][:, wt_ * DC:(wt_ + 1) * DC],
                    start=(p == 0 and cp == 0), stop=(p == 1 and cp == 7),
                    skip_group_check=True)
    for ss in range(4):
        ysb = p3.tile([128, DC], F32, tag="ysb")
        if ss % 2 == 0:
            nc.vector.tensor_copy(ysb[:], yps[ss][:])
        else:
            nc.scalar.copy(ysb[:], yps[ss][:])  # post-p2: Act is free here
        nc.sync.dma_start(
            y[o0 + ss * 128:o0 + (ss + 1) * 128, :], ysb[:])


_PERM = np.concatenate([np.arange(0, 128, 2), np.arange(1, 128, 2)])


def _f16(a):
    return np.ascontiguousarray(a).astype(np.float16)


def make_in_maps(x, wq, wk, wv, wo, freqs_cos, freqs_sin):
    """Host-side sharding/prep. Returns list of 8 per-core input dicts."""
    cosT = freqs_cos.T.astype(np.float16)                         # [64, S]
    sinT = freqs_sin.T.astype(np.float16)
    c2 = np.concatenate([cosT, cosT], axis=0)                     # [128, S]
    s2sw = np.concatenate([sinT, -sinT], axis=0)
    tri = np.where(np.arange(128)[None, :] >= np.arange(128)[:, None],
                   0.0, NEG).astype(np.float16)                   # [k, q]
    ones = np.ones((128, 128), dtype=np.float16)
    idr = np.eye(128, dtype=np.float16)
    trid = np.concatenate([np.full((128, 128), NEG, np.float16),
                           np.where(np.arange(128)[None, :] >=
                                    np.arange(128)[:, None], 0.0,
                                    NEG).astype(np.float16)], axis=1)

    in_maps = []
    for c in range(8):
        b, g = divmod(c, 4)
        cols = slice(g * DC, (g + 1) * DC)
        wq_c = np.ascontiguousarray(wq[:, cols]).copy()
        wk_c = np.ascontiguousarray(wk[:, cols]).copy()
        for h in range(HPC):
            blk = slice(h * 128, (h + 1) * 128)
            wq_c[:, blk] = wq_c[:, blk][:, _PERM]
            wk_c[:, blk] = wk_c[:, blk][:, _PERM]
        in_maps.append({
            "xt": _f16(x[b].T),
            "wq": _f16(wq_c),
            "wk": _f16(wk_c),
            "wv": _f16(wv[:, cols]),
            "wo": _f16(wo[:, cols]),
            "c2": c2, "s2sw": s2sw, "tri": tri, "ones": ones,
            "idr": idr, "trid": trid,
        })
    return in_maps


def assemble(results):
    """Concatenate per-core column outputs into [B, S, D]."""
    out = np.empty((B, S, D), dtype=np.float32)
    for c in range(8):
        b, g = divmod(c, 4)
        out[b][:, g * DC:(g + 1) * DC] = results[c]["y"]
    return out


_NC = None


def kernel(x, wq, wk, wv, wo, freqs_cos, freqs_sin):
    global _NC
    x = np.asarray(x); wq = np.asarray(wq); wk = np.asarray(wk)
    wv = np.asarray(wv); wo = np.asarray(wo)
    freqs_cos = np.asarray(freqs_cos); freqs_sin = np.asarray(freqs_sin)
    if _NC is None:
        _NC = build_module()
    in_maps = make_in_maps(x, wq, wk, wv, wo, freqs_cos, freqs_sin)
    res = run_bass_kernel_spmd(_NC, in_maps, core_ids=list(range(8)))
    return assemble(res.results)


# revision 4
# speedup vs baseline: 22.2994x; 9.5817x over previous
"""Multi-head causal attention with RoPE on 8 trn2 NeuronCores (v2).

Problem (hardcoded): B=2, S=2048, D=2048, H=16, Hd=128, fp32.
  q/k/v = x @ wq/wk/wv; RoPE(q,k); causal softmax(q k^T/sqrt(Hd)) @ v; out @ wo.

Sharding: core c = 4*b + g handles batch b, heads [4g, 4g+4).
Changes vs v1 (see git history in docstring of kernel.py):
  - x is transposed on the HOST (xt = x[b].T, fp16): kills all 64 PE
    transposes, the xt assembly copies, and the xrow DMAs.
  - wq/wk/wv/wo are fully SBUF-resident, DMA'd once at t=0 in 4-chunk
    groups (vs re-streamed every s-block): -18MB HBM traffic, -176 DMA
    dispatch stubs.
  - Causal masks are fp16 (NEG=-30000) instead of fp32r: mask matmuls
    run 1 cycle/row instead of 4 (sub-256-wide fp32r penalty).
  - RoPE runs fully in fp16 on the DVE at the 2x perf mode; the
    partition swap is done with output-base-offset tensor_tensor ops
    (hw-verified) instead of SBUF-to-SBUF DMAs: no Pool-queue load.
  - ep (exp of scores) is fp16; eac (softmax denominator accumulator)
    is fp16 at DVE 2x / first-copy 4x.
  - ysb copies split between DVE and Act to balance engine queues.
"""
import math
import numpy as np

import concourse.bass as bass
import concourse.tile as tile
from concourse import bacc, mybir

F32 = mybir.dt.float32
F32R = mybir.dt.float32r
F16 = mybir.dt.float16
EXPF = mybir.ActivationFunctionType.Exp
ADD = mybir.AluOpType.add
MULT = mybir.AluOpType.mult

B, S, D = 2, 2048, 2048
H, HD = 16, 128
HPC = 4              # heads per core
DC = HPC * HD        # 512 d_out per core
NCHUNK = D // 128    # 16 contraction chunks
NG = NCHUNK // 4     # 4 chunk-groups of 4
SB = 512             # s-block (projection and q-block granularity)
NSB = S // SB        # 4
SCALE = 1.0 / math.sqrt(HD)
NEG = -30000.0       # fp16-safe; exp(scale*(s+NEG)) == 0

RG = [[0, 1, 2, 3], [4, 5, 6, 7]]


def build_module(trace_sim=False, phases=(1, 2, 3), repeat=1):
    nc = bacc.Bacc("TRN2", target_bir_lowering=False, debug=False, num_devices=8)

    xt = nc.dram_tensor("xt", [D, S], F16, kind="ExternalInput").ap()
    wq = nc.dram_tensor("wq", [D, DC], F16, kind="ExternalInput").ap()
    wk = nc.dram_tensor("wk", [D, DC], F16, kind="ExternalInput").ap()
    wv = nc.dram_tensor("wv", [D, DC], F16, kind="ExternalInput").ap()
    wo = nc.dram_tensor("wo", [D, DC], F16, kind="ExternalInput").ap()
    c2 = nc.dram_tensor("c2", [128, S], F16, kind="ExternalInput").ap()
    s2sw = nc.dram_tensor("s2sw", [128, S], F16, kind="ExternalInput").ap()
    tri = nc.dram_tensor("tri", [128, 128], F16, kind="ExternalInput").ap()
    trid = nc.dram_tensor("trid", [128, 256], F16, kind="ExternalInput").ap()
    idr = nc.dram_tensor("idr", [128, 128], F16, kind="ExternalInput").ap()
    ones = nc.dram_tensor("ones", [128, 128], F16, kind="ExternalInput").ap()
    y = nc.dram_tensor("y", [S, DC], F32, kind="ExternalOutput").ap()

    ot_loc = [nc.dram_tensor(f"ot_loc{i}", [DC, SB], F16) for i in range(4)]
    ot_full = [nc.dram_tensor(f"ot_full{i}", [D, SB], F16) for i in range(3)]
    # quarter 3 is gathered in two head-pair halves so the first AllGather
    # hides under the tail of p2(3): piece rows = [core][head 2h..2h+2][hd]
    ot_f3 = [nc.dram_tensor(f"ot_f3{p}", [D // 2, SB], F16) for p in range(2)]

    with tile.TileContext(nc, trace_sim=trace_sim) as tc:
        with tc.tile_pool(name="consts", bufs=1) as cpool:
            # resident weights, 4-chunk groups: w4[g][:, t*DC + dc] is
            # chunk 4g+t. DMA issuance is deferred into the rep loop so
            # the j=0 x tiles can be first in every queue's FIFO.
            def wload(ap_, name, engine, engine0=None):
                tiles = []
                for g in range(NG):
                    t_ = cpool.tile([128, 4 * DC], F16, name=f"{name}{g}")
                    src = ap_[g * 512:(g + 1) * 512, :].rearrange(
                        "(t p) d -> p t d", t=4)
                    dst = t_[:].rearrange("p (t d) -> p t d", t=4)
                    eng = engine0 if (g == 0 and engine0 is not None) else engine
                    eng.dma_start(dst, src)
                    tiles.append(t_)
                return tiles

            def load_consts():
                c2_t = cpool.tile([128, S], F16)
                nc.scalar.dma_start(c2_t[:], c2[:])
                s2_t = cpool.tile([128, S], F16)
                nc.scalar.dma_start(s2_t[:], s2sw[:])
                tri_t = cpool.tile([128, 128], F16)
                nc.gpsimd.dma_start(tri_t[:], tri[:])
                trid_t = cpool.tile([128, 256], F16)
                nc.gpsimd.dma_start(trid_t[:], trid[:])
                idr_t = cpool.tile([128, 128], F16)
                nc.gpsimd.dma_start(idr_t[:], idr[:])
                ones_t = cpool.tile([128, 128], F16)
                nc.gpsimd.dma_start(ones_t[:], ones[:])
                return dict(ones_t=ones_t, tri_t=tri_t, c2_t=c2_t,
                            s2_t=s2_t, idr_t=idr_t, trid_t=trid_t)

            cst = wq_t = wk_t = wv_t = wo_t = None
            for rep in range(repeat):
                with tc.tile_pool(name=f"qkres{rep}", bufs=1) as qkpool, \
                     tc.tile_pool(name=f"vres{rep}", bufs=1) as vpool, \
                     tc.tile_pool(name=f"p1sb{rep}", bufs=2) as p1, \
                     tc.tile_pool(name=f"p1xt{rep}", bufs=2) as p1x, \
                     tc.tile_pool(name=f"p2sb{rep}", bufs=3) as p2, \
                     tc.tile_pool(name=f"p3sb{rep}", bufs=2) as p3, \
                     tc.tile_pool(name=f"ps{rep}", bufs=1, space="PSUM") as ps:
                    qt_res = [qkpool.tile([128, S], F16, name=f"qt{h}")
                              for h in range(HPC)]
                    kt_res = [qkpool.tile([128, S], F16, name=f"kt{h}")
                              for h in range(HPC)]
                    v_t = [vpool.tile([128, DC], F16, name=f"v{kb}")
                           for kb in range(S // 128)]

                    # j=0 x tiles first in the sync/scalar FIFOs, then the
                    # resident weights (wq earliest: the q-pass streams it
                    # chunk-group by chunk-group), then everything else.
                    xt_pre = _xload(nc, 0, xt, p1x)
                    if rep == 0:
                        wq_t = wload(wq, "wq", nc.sync, engine0=nc.gpsimd)
                        cst = load_consts()
                        wk_t = wload(wk, "wk", nc.scalar)
                        wv_t = wload(wv, "wv", nc.gpsimd)
                        wo_t = wload(wo, "wo", nc.gpsimd)

                    for j in range(NSB):
                        _p1_block(nc, j, xt, wq_t, wk_t, wv_t, v_t,
                                  qt_res, kt_res, p1, p1x, ps, cst,
                                  xt_pre if j == 0 else None)
                    def ag(src_ap, dst_ap):
                        nc.gpsimd.collective_compute(
                            "AllGather", mybir.AluOpType.bypass,
                            replica_groups=RG, ins=[src_ap], outs=[dst_ap])

                    if 2 in phases:
                        for j in range(NSB):
                            cb = None
                            if j == 3 and 3 in phases:
                                def cb(hp):
                                    if hp == 0:
                                        ag(ot_loc[3][0:DC // 2, :],
                                           ot_f3[0][:])
                            _p2_block(nc, j, v_t, ot_loc, qt_res, kt_res,
                                      p2, ps, cst, on_hp=cb)
                            if 3 in phases:
                                if j == 3:
                                    _p3_block(nc, 2, ot_full, wo_t, y,
                                              p3, ps)
                                    ag(ot_loc[3][DC // 2:, :], ot_f3[1][:])
                                else:
                                    ag(ot_loc[j][:], ot_full[j][:])
                                if 1 <= j <= 2:
                                    _p3_block(nc, j - 1, ot_full, wo_t, y,
                                              p3, ps)
                        if 3 in phases:
                            _p3_block3(nc, ot_f3, wo_t, y, p3, ps)

    nc.compile()
    return nc


def _xload(nc, j, xt, p1x):
    s0 = j * SB
    xt4 = [p1x.tile([128, 4 * SB], F16, tag=f"xt{g}", name=f"xt_{j}_{g}",
                    bufs=(2 if g < 2 else 1))
           for g in range(NG)]
    for g in range(NG):
        src = xt[g * 512:(g + 1) * 512, s0:s0 + SB].rearrange(
            "(t p) s -> p t s", t=4)
        dst = xt4[g][:].rearrange("p (t s) -> p t s", t=4)
        (nc.sync if g % 2 == 0 else nc.scalar).dma_start(dst, src)
    return xt4


def _p1_block(nc, j, xt, wq_t, wk_t, wv_t, v_t, qt_res, kt_res, p1, p1x, ps,
              cst, xt_pre=None):
    """Projection + RoPE for s-block j from host-transposed x."""
    s0 = j * SB
    xt4 = xt_pre if xt_pre is not None else _xload(nc, j, xt, p1x)

    def xsl(c, lo=0, width=SB):
        g, t = divmod(c, 4)
        return xt4[g][:, t * SB + lo:t * SB + lo + width]

    # q-pass then k-pass: 4 held PSUM accumulators, resident weights
    for (w4, res_list, fam) in ((wq_t, qt_res, "A"), (wk_t, kt_res, "B")):
        prj = [ps.tile([128, SB], F32, tag=f"{fam}{h}", bufs=1,
                       name=f"prj{fam}{j}_{h}") for h in range(HPC)]
        for c in range(NCHUNK):
            g, t = divmod(c, 4)
            for h in range(HPC):
                nc.tensor.matmul(
                    prj[h][:], w4[g][:, t * DC + h * 128:t * DC + (h + 1) * 128],
                    xsl(c), start=(c == 0), stop=(c == NCHUNK - 1))
        for h in range(HPC):
            raw = p1.tile([128, SB], F16, tag="rraw", bufs=2)
            nc.scalar.copy(raw[:], prj[h][:])
            t1 = p1.tile([128, SB], F16, tag="rt1", bufs=2)
            nc.vector.tensor_tensor(
                t1[:], raw[:], cst["c2_t"][:, s0:s0 + SB], op=MULT)
            t2 = p1.tile([128, SB], F16, tag="rt2", bufs=2)
            # partition swap via output-base offset (hw-verified):
            # t2[p] = raw[p^64] * s2sw[p^64]
            nc.vector.tensor_tensor(
                t2[0:64, :], raw[64:128, :], cst["s2_t"][64:128, s0:s0 + SB],
                op=MULT)
            nc.vector.tensor_tensor(
                t2[64:128, :], raw[0:64, :], cst["s2_t"][0:64, s0:s0 + SB],
                op=MULT)
            nc.vector.tensor_tensor(
                res_list[h][:, s0:s0 + SB], t1[:], t2[:], op=ADD)

    # v-pass (natural layout), x-stationary, into resident v_t
    vps = [ps.tile([128, DC], F32, tag=f"A{ss}", bufs=1,
                   name=f"vps{j}_{ss}") for ss in range(SB // 128)]
    for c in range(NCHUNK):
        g, t = divmod(c, 4)
        for ss in range(SB // 128):
            nc.tensor.matmul(
                vps[ss][:], xsl(c, ss * 128, 128),
                wv_t[g][:, t * DC:(t + 1) * DC],
                start=(c == 0), stop=(c == NCHUNK - 1))
    for ss in range(SB // 128):
        nc.vector.tensor_copy(v_t[j * 4 + ss][:], vps[ss][:])


def _p2_block(nc, j, v_t, ot_loc, qt_res, kt_res, p2, ps, cst, on_hp=None):
    """Causal attention for q-block j (keys/values blocks 0..4j+3)."""
    q0 = j * SB
    nkb = 4 * (j + 1)
    groups = [(0, 1), (2, 3)]
    for hp, heads in enumerate(groups):
        pv = {h: ps.tile([128, SB], F32, tag=f"A{h % 2}",
                         name=f"pv{j}_{h}") for h in heads}
        eac = {h: p2.tile([128, SB], F16, tag=f"eac{h % 2}", bufs=2,
                          name=f"eac{j}_{h}") for h in heads}
        # software-pipelined: sc/exp for kb are emitted one iteration
        # ahead of pv(kb), so the in-order PE never sits on an exp wait.
        def lo_of(kb):
            r = kb - 4 * j
            return 0 if r < 0 else (r * 128 if r <= 2 else 256)

        eps = {}
        for kb in range(nkb + 1):
            for h in heads:
                if kb < nkb:
                    lo = lo_of(kb)
                    r = kb - 4 * j
                    # 4-deep sc rotation while B3 is still free (p3 only
                    # occupies yps banks from AG(0) onwards, i.e. j>=2)
                    rot = 4 if j <= 1 else 3
                    sc = ps.tile([128, SB], F32,
                                 tag=f"B{(kb * 2 + h) % rot}",
                                 name=f"sc{j}_{h}_{kb}")
                    nc.tensor.matmul(
                        sc[:, lo:], kt_res[h][:, kb * 128:(kb + 1) * 128],
                        qt_res[h][:, q0 + lo:q0 + SB],
                        start=True, stop=(r < 0), skip_group_check=True)
                    if r == 3:
                        nc.tensor.matmul(
                            sc[:, 256:], cst["idr_t"][:], cst["trid_t"][:],
                            start=False, stop=True, skip_group_check=True)
                    elif r >= 0:
                        nc.tensor.matmul(
                            sc[:, r * 128:(r + 1) * 128], cst["idr_t"][:],
                            cst["tri_t"][:],
                            start=False, stop=True, skip_group_check=True)
                    ep = p2.tile([128, SB], F16, tag="ep", bufs=4)
                    nc.scalar.activation(ep[:, lo:], sc[:, lo:], EXPF,
                                         scale=SCALE)
                    if kb == 0:
                        nc.vector.tensor_copy(eac[h][:], ep[:])
                    else:
                        nc.vector.tensor_tensor(
                            eac[h][:, lo:], eac[h][:, lo:], ep[:, lo:],
                            op=ADD)
                    eps[(kb, h)] = ep
                if kb >= 1:
                    pkb = kb - 1
                    lo = lo_of(pkb)
                    ep = eps.pop((pkb, h))
                    nc.tensor.matmul(
                        pv[h][:, lo:], v_t[pkb][:, h * 128:(h + 1) * 128],
                        ep[:, lo:],
                        start=(pkb == 0), stop=(pkb == nkb - 1),
                        skip_group_check=True)
        for h in heads:
            dn = ps.tile([128, SB], F32, tag="A2", name=f"dn{j}_{h}")
            nc.tensor.matmul(dn[:], cst["ones_t"][:], eac[h][:],
                             start=True, stop=True)
            rec = p2.tile([128, SB], F32, tag="rec", bufs=2)
            nc.vector.reciprocal(rec[:], dn[:])
            ot = p2.tile([128, SB], F16, tag="ot", bufs=2)
            nc.vector.tensor_tensor(ot[:], pv[h][:], rec[:], op=MULT)
            nc.sync.dma_start(
                ot_loc[j][h * 128:(h + 1) * 128, :], ot[:])
        if on_hp is not None:
            on_hp(hp)


def _p3_block(nc, sq, ot_full, wo_t, y, p3, ps):
    """Output projection for q-quarter sq from its gathered o^T."""
    o0 = sq * 512
    otf = [p3.tile([128, 4 * SB], F16, tag=f"otf{g}", bufs=1,
                   name=f"otf{g}_{sq}") for g in range(NG)]
    for g in range(NG):
        src = ot_full[sq][g * 512:(g + 1) * 512, :].rearrange(
            "(t p) s -> p t s", t=4)
        dst = otf[g][:].rearrange("p (t s) -> p t s", t=4)
        # sync only: stubs on the Act queue would delay the p2 exp chain
        nc.sync.dma_start(dst, src)
    for ss in range(4):
        yps = ps.tile([128, DC], F32, tag=("A3" if ss % 2 == 0 else "B3"),
                      name=f"yps{sq}_{ss}")
        for c in range(NCHUNK):
            g, t = divmod(c, 4)
            nc.tensor.matmul(
                yps[:], otf[g][:, t * SB + ss * 128:t * SB + (ss + 1) * 128],
                wo_t[g][:, t * DC:(t + 1) * DC],
                start=(c == 0), stop=(c == NCHUNK - 1))
        ysb = p3.tile([128, DC], F32, tag="ysb")
        nc.vector.tensor_copy(ysb[:], yps[:])
        nc.sync.dma_start(
            y[o0 + ss * 128:o0 + (ss + 1) * 128, :], ysb[:])


def _p3_block3(nc, ot_f3, wo_t, y, p3, ps):
    """Output projection for q-quarter 3 from the two head-pair-half
    gathers. Piece p chunk cp (= core*2 + h-in-pair) maps to original
    o-dim chunk (cp//2)*4 + 2p + cp%2 — a pure index remap on wo_t."""
    o0 = 3 * 512
    otf = {}
    for p in range(2):
        for gg in range(2):
            t_ = p3.tile([128, 4 * SB], F16, tag=f"otf{p * 2 + gg}", bufs=1,
                         name=f"otf3_{p}_{gg}")
            src = ot_f3[p][gg * 512:(gg + 1) * 512, :].rearrange(
                "(t p) s -> p t s", t=4)
            dst = t_[:].rearrange("p (t s) -> p t s", t=4)
            (nc.sync if gg == 0 else nc.scalar).dma_start(dst, src)
            otf[(p, gg)] = t_
    # piece-major: all piece-0 accumulation (hides under the second
    # AllGather), then all piece-1. Four yps banks held concurrently.
    yps = [ps.tile([128, DC], F32, tag=t, name=f"yps3_{i}")
           for i, t in enumerate(("A3", "B3", "A0", "A1"))]
    for p in range(2):
        for ss in range(4):
            for cp in range(8):
                gg, tt = divmod(cp, 4)
                wc = (cp // 2) * 4 + 2 * p + cp % 2
                wg, wt_ = divmod(wc, 4)
                nc.tensor.matmul(
                    yps[ss][:],
                    otf[(p, gg)][:, tt * SB + ss * 128:tt * SB + (ss + 1) * 128],
                    wo_t[wg][:, wt_ * DC:(wt_ + 1) * DC],
                    start=(p == 0 and cp == 0), stop=(p == 1 and cp == 7),
                    skip_group_check=True)
    for ss in range(4):
        ysb = p3.tile([128, DC], F32, tag="ysb")
        if ss % 2 == 0:
            nc.vector.tensor_copy(ysb[:], yps[ss][:])
        else:
            nc.scalar.copy(ysb[:], yps[ss][:])  # post-p2: Act is free here
        nc.sync.dma_start(
            y[o0 + ss * 128:o0 + (ss + 1) * 128, :], ysb[:])


_PERM = np.concatenate([np.arange(0, 128, 2), np.arange(1, 128, 2)])


def _f16(a):
    return np.ascontiguousarray(a).astype(np.float16)


def make_in_maps(x, wq, wk, wv, wo, freqs_cos, freqs_sin):
    """Host-side sharding/prep. Returns list of 8 per-core input dicts."""
    cosT = freqs_cos.T.astype(np.float16)                         # [64, S]
    sinT = freqs_sin.T.astype(np.float16)
    c2 = np.concatenate([cosT, cosT], axis=0)                     # [128, S]
    s2sw = np.concatenate([sinT, -sinT], axis=0)
    tri = np.where(np.arange(128)[None, :] >= np.arange(128)[:, None],
                   0.0, NEG).astype(np.float16)                   # [k, q]
    ones = np.ones((128, 128), dtype=np.float16)
    idr = np.eye(128, dtype=np.float16)
    trid = np.concatenate([np.full((128, 128), NEG, np.float16),
                           np.where(np.arange(128)[None, :] >=
                                    np.arange(128)[:, None], 0.0,
                                    NEG).astype(np.float16)], axis=1)

    in_maps = []
    for c in range(8):
        b, g = divmod(c, 4)
        cols = slice(g * DC, (g + 1) * DC)
        wq_c = np.ascontiguousarray(wq[:, cols]).copy()
        wk_c = np.ascontiguousarray(wk[:, cols]).copy()
        for h in range(HPC):
            blk = slice(h * 128, (h + 1) * 128)
            wq_c[:, blk] = wq_c[:, blk][:, _PERM]
            wk_c[:, blk] = wk_c[:, blk][:, _PERM]
        in_maps.append({
            "xt": _f16(x[b].T),
            "wq": _f16(wq_c),
            "wk": _f16(wk_c),
            "wv": _f16(wv[:, cols]),
            "wo": _f16(wo[:, cols]),
            "c2": c2, "s2sw": s2sw, "tri": tri, "ones": ones,
            "idr": idr, "trid": trid,
        })
    return in_maps


def assemble(results):
    """Concatenate per-core column outputs into [B, S, D]."""
    out = np.empty((B, S, D), dtype=np.float32)
    for c in range(8):
        b, g = divmod(c, 4)
        out[b][:, g * DC:(g + 1) * DC] = results[c]["y"]
    return out


class _Runner:
    """Persistent jitted SPMD executor (mirrors bass2jax.run_bass_via_pjrt
    but caches the jit + module across kernel() calls)."""

    def __init__(self, nc, n_cores=8):
        import jax
        from jax.sharding import Mesh, PartitionSpec, NamedSharding
        from jax.experimental.shard_map import shard_map
        from concourse.bass2jax import (
            _bass_exec_p, install_neuronx_cc_hook, partition_id_tensor)
        install_neuronx_cc_hook()
        self.jax = jax
        self.n_cores = n_cores
        pname = nc.partition_id_tensor.name if nc.partition_id_tensor else None
        in_names, out_names, out_avals, self.out_shapes = [], [], [], []
        for alloc in nc.m.functions[0].allocations:
            if not isinstance(alloc, mybir.MemoryLocationSet):
                continue
            name = alloc.memorylocations[0].name
            if alloc.kind == "ExternalInput":
                if name != pname:
                    in_names.append(name)
            elif alloc.kind == "ExternalOutput":
                out_names.append(name)
                shape = tuple(alloc.tensor_shape)
                dtype = mybir.dt.np(alloc.dtype)
                out_avals.append(jax.core.ShapedArray(shape, dtype))
                self.out_shapes.append((shape, dtype))
        self.in_names = list(in_names)
        self.out_names = out_names
        all_in = in_names + out_names
        if pname is not None:
            all_in.append(pname)

        def _body(*args):
            operands = list(args)
            if pname is not None:
                operands.append(partition_id_tensor())
            return tuple(_bass_exec_p.bind(
                *operands, out_avals=tuple(out_avals),
                in_names=tuple(all_in), out_names=tuple(out_names),
                lowering_input_output_aliases=(),
                sim_require_finite=True, sim_require_nnan=True, nc=nc))

        devices = jax.devices()[:n_cores]
        mesh = Mesh(np.asarray(devices), ("core",))
        spec = PartitionSpec("core")
        n_in = len(in_names) + len(out_names)
        self.sharded = jax.jit(
            shard_map(_body, mesh=mesh, in_specs=(spec,) * n_in,
                      out_specs=(spec,) * len(out_names), check_rep=False),
            donate_argnums=tuple(range(len(in_names), n_in)),
            keep_unused=True)
        self.sharding = NamedSharding(mesh, spec)

    def run(self, in_maps):
        jax = self.jax
        dev_in = [
            jax.device_put(np.concatenate(
                [np.asarray(in_maps[c][name]) for c in range(self.n_cores)],
                axis=0), self.sharding)
            for name in self.in_names
        ]
        zeros = [
            jax.device_put(np.zeros((self.n_cores * s[0], *s[1:]), d),
                           self.sharding)
            for (s, d) in self.out_shapes
        ]
        outs = self.sharded(*dev_in, *zeros)
        return [
            {name: np.asarray(outs[i]).reshape(
                self.n_cores, *self.out_shapes[i][0])[c]
             for i, name in enumerate(self.out_names)}
            for c in range(self.n_cores)
        ]


_RUNNER = None


def kernel(x, wq, wk, wv, wo, freqs_cos, freqs_sin):
    global _RUNNER
    x = np.asarray(x); wq = np.asarray(wq); wk = np.asarray(wk)
    wv = np.asarray(wv); wo = np.asarray(wo)
    freqs_cos = np.asarray(freqs_cos); freqs_sin = np.asarray(freqs_sin)
    if _RUNNER is None:
        _RUNNER = _Runner(build_module())
    in_maps = make_in_maps(x, wq, wk, wv, wo, freqs_cos, freqs_sin)
    return assemble(_RUNNER.run(in_maps))
